# revision 20
# baseline (speedup 1.0000x reference)
"""Trainium2 Bass kernel for width-axis cross attention (sparse_attention problem).

reference semantics:
  Q = conv3x3(low1, w, b); K = conv3x3(low2, w, b)
  score[b,h,w,v] = sum_c Q[b,c,h,w] * K[b,c,h,v]
  A_left  = softmax(score, axis=-1)            (relu is identity on softmax)
  A_right = softmax(score^T, axis=-1)
  left  = low1 + einsum('bhwv,bchv->bchw', A_left,  low2)
  right = low2 + einsum('bhwv,bchv->bchw', A_right, low1)

Sharding: data-parallel over (batch, H-half) -> 8 shards, no cross-core comm.

Per-core dataflow (96 rows, processed in row pairs), fp16 matmul lanes /
fp32 accumulation+normalization:
 - conv as 9 accumulating fp16 matmuls per tensor, 2 output rows per matmul
   (N=384; weight loads amortized and hidden), PSUM -> SBUF fp16 with bias
   via ScalarE.
 - S = Q^T K and St = K^T Q in fp16; exp(S - 12) via ScalarE (one [96,384]
   op per side; the constant shift keeps unnormalized exp inside fp16 range
   and cancels in the softmax normalization).
 - apply matmuls in fp16 against host-pre-transposed inputs, with an extra
   all-ones column producing the softmax row-sums for free (column sums of
   exp land in PSUM column 96).
 - finalize = (M * 1/rs) + base^T in one fused VectorE scalar_tensor_tensor
   op; the fp16 base is re-read from the same xtb tile the apply used, and
   the output is stored width-transposed in fp16 as one tensor (single DMA
   per pair); host un-transposes.
 - the whole padded input block stays resident in SBUF (loaded once in
   row slices, first pair's rows first) -- no per-chunk halo re-reads.

Measured on 8 axon-tunneled TRN2 cores at full clock: HW exec ~231 us
(tensor engine >99% busy at the fp16 streaming bound for its 480k matmul
rows; fp8 fails the accuracy gate, DMA-XBAR transposes trip the activity
throttle), scale-relative max error ~1.4e-3 vs the fp32 reference.
"""

import os
import sys

for _p in ("/opt/trn_rl_repo", "/root/.axon_site/_ro/trn_rl_repo"):
    if os.path.isdir(_p) and _p not in sys.path:
        sys.path.append(_p)

import numpy as np

import concourse.bacc as bacc
import concourse.bass as bass
import concourse.tile as tile
from concourse import mybir
from concourse import bass_utils

B, C, H, W = 4, 96, 192, 192
NCORES = 8
HL = H // 2          # local rows per core
WP = W + 2           # width-padded
WC = W // 2          # 96-wide chunk of the W axis
NPAIR = HL // 2      # 48 row pairs


F32 = mybir.dt.float32
FP16 = mybir.dt.float16
AF = mybir.ActivationFunctionType
ALU = mybir.AluOpType

LP_DT, LP_NP = FP16, np.float16
ESHIFT = 12.0

_CACHE = {}


def _install_profile_hook():
    """Register the axon NTFF profiling hook (missing from this image's antenv)."""
    if _CACHE.get("hook_done"):
        return
    _CACHE["hook_done"] = True
    import types
    import antenv

    if "antenv.axon_hooks" not in sys.modules:
        mod = types.ModuleType("antenv.axon_hooks")
        _h = {"fn": None}
        mod.set_axon_ntff_profile_hook = lambda fn: _h.__setitem__("fn", fn)
        mod.get_axon_ntff_profile_hook = lambda: _h["fn"]
        sys.modules["antenv.axon_hooks"] = mod
        antenv.axon_hooks = mod
    mod = sys.modules["antenv.axon_hooks"]
    try:
        from trn_agent_boot.trn_boot import _ntff_profile_via_ctypes

        hook = _ntff_profile_via_ctypes("/opt/axon/libaxon_pjrt.so")
        if hook is not None:
            mod.set_axon_ntff_profile_hook(hook)
    except Exception as e:  # profiling is best-effort
        print(f"profile hook install failed: {e}", file=sys.stderr)
    # avoid remote artifact uploads from the profiling path
    bass_utils.upload_artifacts = lambda tmpdir: "local://" + str(tmpdir)


def _build():
    """Build + compile the per-core Bass module (identical on all 8 cores)."""
    nc = bacc.Bacc("TRN2", target_bir_lowering=False, debug=False,
                   num_devices=NCORES)

    # inputs (per core); x1/x2 are the padded local row block, loaded once
    # (no per-chunk halo duplication)
    HR = HL + 2   # 98 padded rows resident per tensor
    x1c = nc.dram_tensor("x1c", [C, HR, WP], LP_DT,
                         kind="ExternalInput").ap()
    x2c = nc.dram_tensor("x2c", [C, HR, WP], LP_DT,
                         kind="ExternalInput").ap()
    # [pair, w', row, slot, col]; slots 0,1 = low1T w-chunks, 2,3 = low2T
    # xtb has a 97th all-ones column for free softmax row-sums.
    xtb = nc.dram_tensor("xtb", [NPAIR, WC, 2, 4, WC + 1], LP_DT,
                         kind="ExternalInput").ap()
    wt = nc.dram_tensor("wt", [C, 9, C], LP_DT, kind="ExternalInput").ap()
    bias = nc.dram_tensor("bias", [C, 1], F32, kind="ExternalInput").ap()
    # merged output, transposed layout: [pair, part, side, row, chunk, c]
    oT = nc.dram_tensor("oT", [NPAIR, WC, 2, 2, 2, C], LP_DT,
                        kind="ExternalOutput").ap()

    with tile.TileContext(nc) as tc:
        with (
            tc.tile_pool(name="wpool", bufs=1) as wpool,
            tc.tile_pool(name="xres", bufs=1) as x_pool,
            tc.tile_pool(name="xtbp", bufs=3) as xtb_pool,
            tc.tile_pool(name="qkp", bufs=2) as qk_pool,
            tc.tile_pool(name="ep", bufs=3) as e_pool,
            tc.tile_pool(name="rcpp", bufs=3) as rcp_pool,
            tc.tile_pool(name="outp", bufs=3) as out_pool,
            tc.tile_pool(name="convps", bufs=1, space="PSUM") as conv_pp,
            tc.tile_pool(name="sps", bufs=2, space="PSUM") as s_pp,
            tc.tile_pool(name="stps", bufs=2, space="PSUM") as st_pp,
            tc.tile_pool(name="mps", bufs=2, space="PSUM") as m_pp,
        ):
            # whole padded input block resident in SBUF (38 KB/partition per
            # tensor) -- no per-chunk halo re-reads
            x1_s = x_pool.tile([C, HR, WP], LP_DT)
            x2_s = x_pool.tile([C, HR, WP], LP_DT)
            # first pair's rows land first so the first conv matmul starts
            # as early as possible
            nc.sync.dma_start(x1_s[:, 0:4, :], x1c[:, 0:4, :])
            wt_s = wpool.tile([C, 9, C], LP_DT)
            nc.sync.dma_start(wt_s[:], wt)
            nc.sync.dma_start(x2_s[:, 0:4, :], x2c[:, 0:4, :])
            bias_s = wpool.tile([C, 1], F32)
            nc.sync.dma_start(bias_s[:], bias)
            nc.sync.dma_start(x1_s[:, 4:12, :], x1c[:, 4:12, :])
            nc.sync.dma_start(x2_s[:, 4:12, :], x2c[:, 4:12, :])
            eshift_s = wpool.tile([WC, 1], F32)
            nc.gpsimd.memset(eshift_s[:], -ESHIFT)
            next_row = [12]

            def load_rows():
                lo = next_row[0]
                if lo >= HR:
                    return
                hi = min(lo + 8, HR)
                nc.sync.dma_start(x1_s[:, lo:hi, :], x1c[:, lo:hi, :])
                nc.sync.dma_start(x2_s[:, lo:hi, :], x2c[:, lo:hi, :])
                next_row[0] = hi
            state = {}

            def emit_conv(q):
                """conv3x3 + bias/cast for pair q; prefetch DMAs for pair q."""
                if q % 4 == 0:
                    load_rows()

                xtb_t = xtb_pool.tile([WC, 2, 4, WC + 1], LP_DT)
                nc.sync.dma_start(xtb_t[:], xtb[q])

                q_ps = conv_pp.tile([C, 2, W], F32, tag="q2")
                k_ps = conv_pp.tile([C, 2, W], F32, tag="k2")
                qk = qk_pool.tile([C, 2, 2, W], LP_DT)   # [c, row, q/k, w]
                # all Q taps first: the Q copy starts while K taps stream, and
                # the first pair doesn't stall on the x2 DMA
                for t in range(9):
                    ky, kx = t // 3, t % 3
                    r = 2 * q + ky
                    nc.tensor.matmul(q_ps[:], wt_s[:, t, :],
                                     x1_s[:, r:r + 2, kx:kx + W],
                                     start=(t == 0), stop=(t == 8))
                nc.scalar.activation(qk[:, :, 0, :], q_ps[:], AF.Identity,
                                     bias=bias_s[:], scale=1.0)
                for t in range(9):
                    ky, kx = t // 3, t % 3
                    r = 2 * q + ky
                    nc.tensor.matmul(k_ps[:], wt_s[:, t, :],
                                     x2_s[:, r:r + 2, kx:kx + W],
                                     start=(t == 0), stop=(t == 8))
                nc.scalar.activation(qk[:, :, 1, :], k_ps[:], AF.Identity,
                                     bias=bias_s[:], scale=1.0)
                state[q] = (qk, xtb_t)

            def emit_attn(q):
                """width attention + finalize + store for pair q."""
                qk, xtb_t = state.pop(q)
                out_t = out_pool.tile([WC, 2, 2, 2, C], LP_DT)
                # scores + exp for BOTH rows first: row 1's score matmuls hide
                # row 0's exp latency even when no conv filler is available
                # (pipeline drain at the last pairs).
                e_ts = []
                for rr in range(2):
                    # --- S = Q^T K and St = K^T Q, chunked over W ---
                    s_ps = s_pp.tile([WC, 2, W], F32)
                    st_ps = st_pp.tile([WC, 2, W], F32)
                    for wc in range(2):
                        nc.tensor.matmul(s_ps[:, wc, :],
                                         qk[:, rr, 0, bass.ts(wc, WC)],
                                         qk[:, rr, 1, :],
                                         start=True, stop=True)
                        nc.tensor.matmul(st_ps[:, wc, :],
                                         qk[:, rr, 1, bass.ts(wc, WC)],
                                         qk[:, rr, 0, :],
                                         start=True, stop=True)
                    # --- E = exp(S), Et = exp(St); Et first: the left-apply
                    # (first consumer) reads slots 2,3, so at pipeline drain
                    # its matmuls start while the S exp still runs ---
                    e_t = e_pool.tile([WC, 4, W], LP_DT)
                    nc.scalar.activation(e_t[:, 2:4, :], st_ps[:], AF.Exp,
                                         bias=eshift_s[:])
                    nc.scalar.activation(e_t[:, 0:2, :], s_ps[:], AF.Exp,
                                         bias=eshift_s[:])
                    e_ts.append(e_t)
                for rr in range(2):
                    e_t = e_ts[rr]
                    # --- apply (unnormalized) + ones-column row-sums ---
                    m_ps = m_pp.tile([WC, 4, WC + 1], F32)
                    for wc in range(2):
                        for vc in range(2):
                            nc.tensor.matmul(
                                m_ps[:, wc, :],
                                e_t[:, 2 + vc, bass.ts(wc, WC)],
                                xtb_t[:, rr, 2 + vc, :],
                                start=(vc == 0), stop=(vc == 1))
                    for vc in range(2):
                        for wc in range(2):
                            nc.tensor.matmul(
                                m_ps[:, 2 + vc, :],
                                e_t[:, wc, bass.ts(vc, WC)],
                                xtb_t[:, rr, wc, :],
                                start=(wc == 0), stop=(wc == 1))

                    rcp_t = rcp_pool.tile([WC, 4], F32)
                    nc.vector.reciprocal(rcp_t[:], m_ps[:, :, WC:WC + 1])

                    # --- finalize: out = base^T + M * (1/rs) ---
                    for wc in range(2):
                        nc.vector.scalar_tensor_tensor(
                            out_t[:, 0, rr, wc, :], m_ps[:, wc, 0:C],
                            rcp_t[:, wc:wc + 1], xtb_t[:, rr, wc, 0:C],
                            op0=ALU.mult, op1=ALU.add)
                        nc.vector.scalar_tensor_tensor(
                            out_t[:, 1, rr, wc, :], m_ps[:, 2 + wc, 0:C],
                            rcp_t[:, 2 + wc:3 + wc], xtb_t[:, rr, 2 + wc, 0:C],
                            op0=ALU.mult, op1=ALU.add)
                nc.sync.dma_start(oT[q], out_t[:])

            # software pipeline: conv runs one pair ahead of attention
            emit_conv(0)
            for q in range(NPAIR):
                if q + 1 < NPAIR:
                    emit_conv(q + 1)
                emit_attn(q)

    nc.compile()
    return nc


def _prepare_inputs(low1, low2, conv_w, conv_b):
    low1 = np.asarray(low1, dtype=np.float32)
    low2 = np.asarray(low2, dtype=np.float32)
    conv_w = np.asarray(conv_w, dtype=np.float32)
    conv_b = np.asarray(conv_b, dtype=np.float32)

    xp1 = np.zeros((B, C, H + 2, W + 2), np.float32)
    xp1[:, :, 1:-1, 1:-1] = low1
    xp2 = np.zeros((B, C, H + 2, W + 2), np.float32)
    xp2[:, :, 1:-1, 1:-1] = low2

    wt = np.ascontiguousarray(
        conv_w.transpose(1, 2, 3, 0).reshape(C, 9, C)).astype(np.float16)
    bias = np.ascontiguousarray(conv_b.reshape(C, 1))

    in_maps = []
    for k in range(NCORES):
        b, half = k // 2, k % 2
        r0 = half * HL

        # padded local row block [C, HL+2, W+2] per tensor
        x1ck = np.ascontiguousarray(
            xp1[b, :, r0:r0 + HL + 2, :]).astype(np.float16)
        x2ck = np.ascontiguousarray(
            xp2[b, :, r0:r0 + HL + 2, :]).astype(np.float16)

        # transposed [h, w', slot, c] for both tensors; slot 0,1=low1T, 2,3=low2T
        l1t = low1[b, :, r0:r0 + HL, :].transpose(1, 2, 0)   # [h, w, c]
        l2t = low2[b, :, r0:r0 + HL, :].transpose(1, 2, 0)
        a1 = l1t.reshape(HL, 2, WC, C).transpose(0, 2, 1, 3)  # [h, w', wc, c]
        a2 = l2t.reshape(HL, 2, WC, C).transpose(0, 2, 1, 3)
        xt = np.concatenate([a1, a2], axis=2)                 # [h, w', 4, c]
        # pair-batch: [pair, w', row, slot, c] + ones column
        xt32 = xt.reshape(NPAIR, 2, WC, 4, C).transpose(0, 2, 1, 3, 4)
        xtb = np.concatenate(
            [xt32, np.ones((NPAIR, WC, 2, 4, 1), np.float32)],
            axis=4).astype(np.float16)

        in_maps.append({
            "x1c": x1ck,
            "x2c": x2ck,
            "xtb": np.ascontiguousarray(xtb),
            "wt": wt,
            "bias": bias,
        })
    return in_maps


def _assemble(results):
    left = np.empty((B, C, H, W), np.float32)
    right = np.empty((B, C, H, W), np.float32)
    for k in range(NCORES):
        b, half = k // 2, k % 2
        r0 = half * HL
        arr = results[k]["oT"].astype(np.float32)  # [pair, p, side, rr, wc, c]
        for side, dst in ((0, left), (1, right)):
            # -> [c, pair, rr, wc, p] -> [c, h, w]
            dst[b, :, r0:r0 + HL, :] = (
                arr[:, :, side].transpose(4, 0, 2, 3, 1).reshape(C, HL, W))
    return left, right


def _run(inputs, trace=False):
    if trace:
        _install_profile_hook()
    if "nc" not in _CACHE:
        _CACHE["nc"] = _build()
    nc = _CACHE["nc"]
    in_maps = _prepare_inputs(**inputs)
    res = bass_utils.run_bass_kernel_spmd(
        nc, in_maps, core_ids=list(range(NCORES)), trace=trace)
    left, right = _assemble(res.results)
    return (left, right), res


def kernel(**inputs):
    out, _ = _run(inputs, trace=False)
    return out


# revision 22
# speedup vs baseline: 1.1915x; 1.1915x over previous
"""Trainium2 Bass kernel for width-axis cross attention (sparse_attention problem).

reference semantics:
  Q = conv3x3(low1, w, b); K = conv3x3(low2, w, b)
  score[b,h,w,v] = sum_c Q[b,c,h,w] * K[b,c,h,v]
  A_left  = softmax(score, axis=-1)            (relu is identity on softmax)
  A_right = softmax(score^T, axis=-1)
  left  = low1 + einsum('bhwv,bchv->bchw', A_left,  low2)
  right = low2 + einsum('bhwv,bchv->bchw', A_right, low1)

Sharding: data-parallel over (batch, H-half) -> 8 shards, no cross-core comm.

Per-core dataflow (96 rows, processed in row pairs), fp16 matmul lanes /
fp32 accumulation+normalization:
 - conv as 9 accumulating fp16 matmuls per tensor, 2 output rows per matmul
   (N=384; weight loads amortized and hidden), PSUM -> SBUF fp16 with bias
   via ScalarE.
 - S = Q^T K and St = K^T Q in fp16; exp(S - 12) via ScalarE (one [96,384]
   op per side; the constant shift keeps unnormalized exp inside fp16 range
   and cancels in the softmax normalization).
 - apply matmuls in fp16 against host-pre-transposed inputs, with an extra
   all-ones column producing the softmax row-sums for free (column sums of
   exp land in PSUM column 96).
 - finalize = (M * 1/rs) + base^T in one fused VectorE scalar_tensor_tensor
   op; the fp16 base is re-read from the same xtb tile the apply used, and
   the output is stored width-transposed in fp16 as one tensor (single DMA
   per pair); host un-transposes.
 - the whole padded input block stays resident in SBUF (loaded once in
   row slices, first pair's rows first) -- no per-chunk halo re-reads.

Measured on 8 axon-tunneled TRN2 cores at full clock: HW exec ~231 us
(tensor engine >99% busy at the fp16 streaming bound for its 480k matmul
rows; fp8 fails the accuracy gate, DMA-XBAR transposes trip the activity
throttle), scale-relative max error ~1.4e-3 vs the fp32 reference.
"""

import os
import sys

for _p in ("/opt/trn_rl_repo", "/root/.axon_site/_ro/trn_rl_repo"):
    if os.path.isdir(_p) and _p not in sys.path:
        sys.path.append(_p)

import numpy as np

import concourse.bacc as bacc
import concourse.bass as bass
import concourse.tile as tile
from concourse import mybir
from concourse import bass_utils

B, C, H, W = 4, 96, 192, 192
NCORES = 8
HL = H // 2          # local rows per core
WP = W + 2           # width-padded
WC = W // 2          # 96-wide chunk of the W axis
NPAIR = HL // 2      # 48 row pairs


F32 = mybir.dt.float32
FP16 = mybir.dt.float16
AF = mybir.ActivationFunctionType
ALU = mybir.AluOpType

LP_DT, LP_NP = FP16, np.float16
ESHIFT = 12.0

_CACHE = {}


def _install_profile_hook():
    """Register the axon NTFF profiling hook (missing from this image's antenv)."""
    if _CACHE.get("hook_done"):
        return
    _CACHE["hook_done"] = True
    import types
    import antenv

    if "antenv.axon_hooks" not in sys.modules:
        mod = types.ModuleType("antenv.axon_hooks")
        _h = {"fn": None}
        mod.set_axon_ntff_profile_hook = lambda fn: _h.__setitem__("fn", fn)
        mod.get_axon_ntff_profile_hook = lambda: _h["fn"]
        sys.modules["antenv.axon_hooks"] = mod
        antenv.axon_hooks = mod
    mod = sys.modules["antenv.axon_hooks"]
    try:
        from trn_agent_boot.trn_boot import _ntff_profile_via_ctypes

        hook = _ntff_profile_via_ctypes("/opt/axon/libaxon_pjrt.so")
        if hook is not None:
            mod.set_axon_ntff_profile_hook(hook)
    except Exception as e:  # profiling is best-effort
        print(f"profile hook install failed: {e}", file=sys.stderr)
    # avoid remote artifact uploads from the profiling path
    bass_utils.upload_artifacts = lambda tmpdir: "local://" + str(tmpdir)


def _build():
    """Build + compile the per-core Bass module (identical on all 8 cores)."""
    nc = bacc.Bacc("TRN2", target_bir_lowering=False, debug=False,
                   num_devices=NCORES)

    # inputs (per core); x1/x2 are the padded local row block, loaded once
    # (no per-chunk halo duplication)
    HR = HL + 2   # 98 padded rows resident per tensor
    x1c = nc.dram_tensor("x1c", [C, HR, WP], LP_DT,
                         kind="ExternalInput").ap()
    x2c = nc.dram_tensor("x2c", [C, HR, WP], LP_DT,
                         kind="ExternalInput").ap()
    # [pair, w', row, slot, col]; slots 0,1 = low1T w-chunks, 2,3 = low2T
    # xtb has a 97th all-ones column for free softmax row-sums.
    xtb = nc.dram_tensor("xtb", [NPAIR, WC, 2, 4, WC + 1], LP_DT,
                         kind="ExternalInput").ap()
    wt = nc.dram_tensor("wt", [C, 9, C], LP_DT, kind="ExternalInput").ap()
    bias = nc.dram_tensor("bias", [C, 1], F32, kind="ExternalInput").ap()
    # merged output, transposed layout: [pair, part, side, row, chunk, c]
    oT = nc.dram_tensor("oT", [NPAIR, WC, 2, 2, 2, C], LP_DT,
                        kind="ExternalOutput").ap()

    with tile.TileContext(nc) as tc:
        with (
            tc.tile_pool(name="wpool", bufs=1) as wpool,
            tc.tile_pool(name="xres", bufs=1) as x_pool,
            tc.tile_pool(name="xtbp", bufs=3) as xtb_pool,
            tc.tile_pool(name="qkp", bufs=2) as qk_pool,
            tc.tile_pool(name="ep", bufs=3) as e_pool,
            tc.tile_pool(name="rcpp", bufs=3) as rcp_pool,
            tc.tile_pool(name="outp", bufs=3) as out_pool,
            tc.tile_pool(name="convps", bufs=1, space="PSUM") as conv_pp,
            tc.tile_pool(name="sps", bufs=2, space="PSUM") as s_pp,
            tc.tile_pool(name="stps", bufs=2, space="PSUM") as st_pp,
            tc.tile_pool(name="mps", bufs=2, space="PSUM") as m_pp,
        ):
            # whole padded input block resident in SBUF (38 KB/partition per
            # tensor) -- no per-chunk halo re-reads
            x1_s = x_pool.tile([C, HR, WP], LP_DT)
            x2_s = x_pool.tile([C, HR, WP], LP_DT)
            # first pair's rows land first, channel-split across queues, so
            # the first conv matmul starts as early as possible (taps 0-5
            # only read rows 0:3; the weights gate the first LDWEIGHTS)
            wt_s = wpool.tile([C, 9, C], LP_DT)
            nc.sync.dma_start(wt_s[0:48, :, :], wt[0:48])
            nc.sync.dma_start(wt_s[48:C, :, :], wt[48:C])
            nc.sync.dma_start(x1_s[0:48, 0:3, :], x1c[0:48, 0:3, :])
            nc.sync.dma_start(x1_s[48:C, 0:3, :], x1c[48:C, 0:3, :])
            nc.sync.dma_start(x1_s[:, 3:4, :], x1c[:, 3:4, :])
            nc.sync.dma_start(x2_s[:, 0:4, :], x2c[:, 0:4, :])
            bias_s = wpool.tile([C, 1], F32)
            nc.sync.dma_start(bias_s[:], bias)
            nc.sync.dma_start(x1_s[:, 4:12, :], x1c[:, 4:12, :])
            nc.sync.dma_start(x2_s[:, 4:12, :], x2c[:, 4:12, :])
            eshift_s = wpool.tile([WC, 1], F32)
            nc.gpsimd.memset(eshift_s[:], -ESHIFT)
            next_row = [12]

            def load_rows():
                lo = next_row[0]
                if lo >= HR:
                    return
                hi = min(lo + 8, HR)
                nc.sync.dma_start(x1_s[:, lo:hi, :], x1c[:, lo:hi, :])
                nc.sync.dma_start(x2_s[:, lo:hi, :], x2c[:, lo:hi, :])
                next_row[0] = hi
            state = {}

            def emit_conv(q):
                """conv3x3 + bias/cast for pair q; prefetch DMAs for pair q."""
                if q % 4 == 0:
                    load_rows()

                xtb_t = xtb_pool.tile([WC, 2, 4, WC + 1], LP_DT)
                nc.sync.dma_start(xtb_t[:], xtb[q])

                q_ps = conv_pp.tile([C, 2, W], F32, tag="q2")
                k_ps = conv_pp.tile([C, 2, W], F32, tag="k2")
                qk = qk_pool.tile([C, 2, 2, W], LP_DT)   # [c, row, q/k, w]
                # all Q taps first: the Q copy starts while K taps stream, and
                # the first pair doesn't stall on the x2 DMA
                for t in range(9):
                    ky, kx = t // 3, t % 3
                    r = 2 * q + ky
                    nc.tensor.matmul(q_ps[:], wt_s[:, t, :],
                                     x1_s[:, r:r + 2, kx:kx + W],
                                     start=(t == 0), stop=(t == 8))
                nc.scalar.activation(qk[:, :, 0, :], q_ps[:], AF.Identity,
                                     bias=bias_s[:], scale=1.0)
                for t in range(9):
                    ky, kx = t // 3, t % 3
                    r = 2 * q + ky
                    nc.tensor.matmul(k_ps[:], wt_s[:, t, :],
                                     x2_s[:, r:r + 2, kx:kx + W],
                                     start=(t == 0), stop=(t == 8))
                nc.scalar.activation(qk[:, :, 1, :], k_ps[:], AF.Identity,
                                     bias=bias_s[:], scale=1.0)
                state[q] = (qk, xtb_t)

            def emit_attn(q):
                """width attention + finalize + store for pair q."""
                qk, xtb_t = state.pop(q)
                out_t = out_pool.tile([WC, 2, 2, 2, C], LP_DT)
                # scores + exp for BOTH rows first: row 1's score matmuls hide
                # row 0's exp latency even when no conv filler is available
                # (pipeline drain at the last pairs).
                e_ts = []
                for rr in range(2):
                    # --- S = Q^T K and St = K^T Q, chunked over W ---
                    s_ps = s_pp.tile([WC, 2, W], F32)
                    st_ps = st_pp.tile([WC, 2, W], F32)
                    for wc in range(2):
                        nc.tensor.matmul(s_ps[:, wc, :],
                                         qk[:, rr, 0, bass.ts(wc, WC)],
                                         qk[:, rr, 1, :],
                                         start=True, stop=True)
                        nc.tensor.matmul(st_ps[:, wc, :],
                                         qk[:, rr, 1, bass.ts(wc, WC)],
                                         qk[:, rr, 0, :],
                                         start=True, stop=True)
                    # --- E = exp(S), Et = exp(St); Et first: the left-apply
                    # (first consumer) reads slots 2,3, so at pipeline drain
                    # its matmuls start while the S exp still runs ---
                    e_t = e_pool.tile([WC, 4, W], LP_DT)
                    nc.scalar.activation(e_t[:, 2:4, :], st_ps[:], AF.Exp,
                                         bias=eshift_s[:])
                    nc.scalar.activation(e_t[:, 0:2, :], s_ps[:], AF.Exp,
                                         bias=eshift_s[:])
                    e_ts.append(e_t)
                for rr in range(2):
                    e_t = e_ts[rr]
                    # --- apply (unnormalized) + ones-column row-sums ---
                    m_ps = m_pp.tile([WC, 4, WC + 1], F32)
                    for wc in range(2):
                        for vc in range(2):
                            nc.tensor.matmul(
                                m_ps[:, wc, :],
                                e_t[:, 2 + vc, bass.ts(wc, WC)],
                                xtb_t[:, rr, 2 + vc, :],
                                start=(vc == 0), stop=(vc == 1))
                    for vc in range(2):
                        for wc in range(2):
                            nc.tensor.matmul(
                                m_ps[:, 2 + vc, :],
                                e_t[:, wc, bass.ts(vc, WC)],
                                xtb_t[:, rr, wc, :],
                                start=(wc == 0), stop=(wc == 1))

                    rcp_t = rcp_pool.tile([WC, 4], F32)
                    nc.vector.reciprocal(rcp_t[:], m_ps[:, :, WC:WC + 1])

                    # --- finalize: out = base^T + M * (1/rs) ---
                    for wc in range(2):
                        nc.vector.scalar_tensor_tensor(
                            out_t[:, 0, rr, wc, :], m_ps[:, wc, 0:C],
                            rcp_t[:, wc:wc + 1], xtb_t[:, rr, wc, 0:C],
                            op0=ALU.mult, op1=ALU.add)
                        nc.vector.scalar_tensor_tensor(
                            out_t[:, 1, rr, wc, :], m_ps[:, 2 + wc, 0:C],
                            rcp_t[:, 2 + wc:3 + wc], xtb_t[:, rr, 2 + wc, 0:C],
                            op0=ALU.mult, op1=ALU.add)
                if q == NPAIR - 1:
                    # last pair: ship row 0 while row 1 finalizes, halving
                    # the final post-compute transfer
                    nc.sync.dma_start(oT[q][:, :, 0, :, :], out_t[:, :, 0, :, :])
                    nc.sync.dma_start(oT[q][:, :, 1, :, :], out_t[:, :, 1, :, :])
                else:
                    nc.sync.dma_start(oT[q], out_t[:])

            # software pipeline: conv runs one pair ahead of attention
            emit_conv(0)
            for q in range(NPAIR):
                if q + 1 < NPAIR:
                    emit_conv(q + 1)
                emit_attn(q)

    nc.compile()
    return nc


def _prepare_inputs(low1, low2, conv_w, conv_b):
    low1 = np.asarray(low1, dtype=np.float32)
    low2 = np.asarray(low2, dtype=np.float32)
    conv_w = np.asarray(conv_w, dtype=np.float32)
    conv_b = np.asarray(conv_b, dtype=np.float32)

    xp1 = np.zeros((B, C, H + 2, W + 2), np.float32)
    xp1[:, :, 1:-1, 1:-1] = low1
    xp2 = np.zeros((B, C, H + 2, W + 2), np.float32)
    xp2[:, :, 1:-1, 1:-1] = low2

    wt = np.ascontiguousarray(
        conv_w.transpose(1, 2, 3, 0).reshape(C, 9, C)).astype(np.float16)
    bias = np.ascontiguousarray(conv_b.reshape(C, 1))

    in_maps = []
    for k in range(NCORES):
        b, half = k // 2, k % 2
        r0 = half * HL

        # padded local row block [C, HL+2, W+2] per tensor
        x1ck = np.ascontiguousarray(
            xp1[b, :, r0:r0 + HL + 2, :]).astype(np.float16)
        x2ck = np.ascontiguousarray(
            xp2[b, :, r0:r0 + HL + 2, :]).astype(np.float16)

        # transposed [h, w', slot, c] for both tensors; slot 0,1=low1T, 2,3=low2T
        l1t = low1[b, :, r0:r0 + HL, :].transpose(1, 2, 0)   # [h, w, c]
        l2t = low2[b, :, r0:r0 + HL, :].transpose(1, 2, 0)
        a1 = l1t.reshape(HL, 2, WC, C).transpose(0, 2, 1, 3)  # [h, w', wc, c]
        a2 = l2t.reshape(HL, 2, WC, C).transpose(0, 2, 1, 3)
        xt = np.concatenate([a1, a2], axis=2)                 # [h, w', 4, c]
        # pair-batch: [pair, w', row, slot, c] + ones column
        xt32 = xt.reshape(NPAIR, 2, WC, 4, C).transpose(0, 2, 1, 3, 4)
        xtb = np.concatenate(
            [xt32, np.ones((NPAIR, WC, 2, 4, 1), np.float32)],
            axis=4).astype(np.float16)

        in_maps.append({
            "x1c": x1ck,
            "x2c": x2ck,
            "xtb": np.ascontiguousarray(xtb),
            "wt": wt,
            "bias": bias,
        })
    return in_maps


def _assemble(results):
    left = np.empty((B, C, H, W), np.float32)
    right = np.empty((B, C, H, W), np.float32)
    for k in range(NCORES):
        b, half = k // 2, k % 2
        r0 = half * HL
        arr = results[k]["oT"].astype(np.float32)  # [pair, p, side, rr, wc, c]
        for side, dst in ((0, left), (1, right)):
            # -> [c, pair, rr, wc, p] -> [c, h, w]
            dst[b, :, r0:r0 + HL, :] = (
                arr[:, :, side].transpose(4, 0, 2, 3, 1).reshape(C, HL, W))
    return left, right


def _run(inputs, trace=False):
    if trace:
        _install_profile_hook()
    if "nc" not in _CACHE:
        _CACHE["nc"] = _build()
    nc = _CACHE["nc"]
    in_maps = _prepare_inputs(**inputs)
    res = bass_utils.run_bass_kernel_spmd(
        nc, in_maps, core_ids=list(range(NCORES)), trace=trace)
    left, right = _assemble(res.results)
    return (left, right), res


def kernel(**inputs):
    out, _ = _run(inputs, trace=False)
    return out


# revision 23
# speedup vs baseline: 1.1944x; 1.0024x over previous
"""Trainium2 Bass kernel for width-axis cross attention (sparse_attention problem).

reference semantics:
  Q = conv3x3(low1, w, b); K = conv3x3(low2, w, b)
  score[b,h,w,v] = sum_c Q[b,c,h,w] * K[b,c,h,v]
  A_left  = softmax(score, axis=-1)            (relu is identity on softmax)
  A_right = softmax(score^T, axis=-1)
  left  = low1 + einsum('bhwv,bchv->bchw', A_left,  low2)
  right = low2 + einsum('bhwv,bchv->bchw', A_right, low1)

Sharding: data-parallel over (batch, H-half) -> 8 shards, no cross-core comm.

Per-core dataflow (96 rows, processed in row pairs), fp16 matmul lanes /
fp32 accumulation+normalization:
 - conv as 9 accumulating fp16 matmuls per tensor, 2 output rows per matmul
   (N=384; weight loads amortized and hidden), PSUM -> SBUF fp16 with bias
   via ScalarE.
 - S = Q^T K and St = K^T Q in fp16; exp(S - 12) via ScalarE (one [96,384]
   op per side; the constant shift keeps unnormalized exp inside fp16 range
   and cancels in the softmax normalization).
 - apply matmuls in fp16 against host-pre-transposed inputs, with an extra
   all-ones column producing the softmax row-sums for free (column sums of
   exp land in PSUM column 96).
 - finalize = (M * 1/rs) + base^T in one fused VectorE scalar_tensor_tensor
   op; the fp16 base is re-read from the same xtb tile the apply used, and
   the output is stored width-transposed in fp16 as one tensor (single DMA
   per pair); host un-transposes.
 - the whole padded input block stays resident in SBUF (loaded once in
   row slices, first pair's rows first) -- no per-chunk halo re-reads.

Measured on 8 axon-tunneled TRN2 cores at full clock: HW exec ~231 us
(tensor engine >99% busy at the fp16 streaming bound for its 480k matmul
rows; fp8 fails the accuracy gate, DMA-XBAR transposes trip the activity
throttle), scale-relative max error ~1.4e-3 vs the fp32 reference.
"""

import os
import sys

for _p in ("/opt/trn_rl_repo", "/root/.axon_site/_ro/trn_rl_repo"):
    if os.path.isdir(_p) and _p not in sys.path:
        sys.path.append(_p)

import numpy as np

import concourse.bacc as bacc
import concourse.bass as bass
import concourse.tile as tile
from concourse import mybir
from concourse import bass_utils

B, C, H, W = 4, 96, 192, 192
NCORES = 8
HL = H // 2          # local rows per core
WP = W + 2           # width-padded
WC = W // 2          # 96-wide chunk of the W axis
NPAIR = HL // 2      # 48 row pairs


F32 = mybir.dt.float32
FP16 = mybir.dt.float16
AF = mybir.ActivationFunctionType
ALU = mybir.AluOpType

LP_DT, LP_NP = FP16, np.float16
ESHIFT = 12.0

_CACHE = {}


def _install_profile_hook():
    """Register the axon NTFF profiling hook (missing from this image's antenv)."""
    if _CACHE.get("hook_done"):
        return
    _CACHE["hook_done"] = True
    import types
    import antenv

    if "antenv.axon_hooks" not in sys.modules:
        mod = types.ModuleType("antenv.axon_hooks")
        _h = {"fn": None}
        mod.set_axon_ntff_profile_hook = lambda fn: _h.__setitem__("fn", fn)
        mod.get_axon_ntff_profile_hook = lambda: _h["fn"]
        sys.modules["antenv.axon_hooks"] = mod
        antenv.axon_hooks = mod
    mod = sys.modules["antenv.axon_hooks"]
    try:
        from trn_agent_boot.trn_boot import _ntff_profile_via_ctypes

        hook = _ntff_profile_via_ctypes("/opt/axon/libaxon_pjrt.so")
        if hook is not None:
            mod.set_axon_ntff_profile_hook(hook)
    except Exception as e:  # profiling is best-effort
        print(f"profile hook install failed: {e}", file=sys.stderr)
    # avoid remote artifact uploads from the profiling path
    bass_utils.upload_artifacts = lambda tmpdir: "local://" + str(tmpdir)


def _build():
    """Build + compile the per-core Bass module (identical on all 8 cores)."""
    nc = bacc.Bacc("TRN2", target_bir_lowering=False, debug=False,
                   num_devices=NCORES)

    # inputs (per core); x1/x2 are the padded local row block, loaded once
    # (no per-chunk halo duplication)
    HR = HL + 2   # 98 padded rows resident per tensor
    x1c = nc.dram_tensor("x1c", [C, HR, WP], LP_DT,
                         kind="ExternalInput").ap()
    x2c = nc.dram_tensor("x2c", [C, HR, WP], LP_DT,
                         kind="ExternalInput").ap()
    # [pair, w', row, slot, col]; slots 0,1 = low1T w-chunks, 2,3 = low2T
    # xtb has a 97th all-ones column for free softmax row-sums.
    xtb = nc.dram_tensor("xtb", [NPAIR, WC, 2, 4, WC + 1], LP_DT,
                         kind="ExternalInput").ap()
    wt = nc.dram_tensor("wt", [C, 9, C], LP_DT, kind="ExternalInput").ap()
    bias = nc.dram_tensor("bias", [C, 1], F32, kind="ExternalInput").ap()
    # merged output, transposed layout: [pair, part, side, row, chunk, c]
    oT = nc.dram_tensor("oT", [NPAIR, WC, 2, 2, 2, C], LP_DT,
                        kind="ExternalOutput").ap()

    with tile.TileContext(nc) as tc:
        with (
            tc.tile_pool(name="wpool", bufs=1) as wpool,
            tc.tile_pool(name="xres", bufs=1) as x_pool,
            tc.tile_pool(name="xtbp", bufs=3) as xtb_pool,
            tc.tile_pool(name="qkp", bufs=2) as qk_pool,
            tc.tile_pool(name="ep", bufs=3) as e_pool,
            tc.tile_pool(name="rcpp", bufs=3) as rcp_pool,
            tc.tile_pool(name="outp", bufs=3) as out_pool,
            tc.tile_pool(name="convps", bufs=1, space="PSUM") as conv_pp,
            tc.tile_pool(name="sps", bufs=2, space="PSUM") as s_pp,
            tc.tile_pool(name="stps", bufs=2, space="PSUM") as st_pp,
            tc.tile_pool(name="mps", bufs=2, space="PSUM") as m_pp,
        ):
            # whole padded input block resident in SBUF (38 KB/partition per
            # tensor) -- no per-chunk halo re-reads
            x1_s = x_pool.tile([C, HR, WP], LP_DT)
            x2_s = x_pool.tile([C, HR, WP], LP_DT)
            # first pair's rows land first so the first conv matmul starts
            # as early as possible; dispatch cost (~600ns each) means fewer,
            # earlier DMAs beat channel-splitting here
            nc.sync.dma_start(x1_s[:, 0:4, :], x1c[:, 0:4, :])
            wt_s = wpool.tile([C, 9, C], LP_DT)
            nc.sync.dma_start(wt_s[:], wt)
            nc.sync.dma_start(x2_s[:, 0:4, :], x2c[:, 0:4, :])
            bias_s = wpool.tile([C, 1], F32)
            nc.sync.dma_start(bias_s[:], bias)
            nc.sync.dma_start(x1_s[:, 4:12, :], x1c[:, 4:12, :])
            nc.sync.dma_start(x2_s[:, 4:12, :], x2c[:, 4:12, :])
            eshift_s = wpool.tile([WC, 1], F32)
            nc.gpsimd.memset(eshift_s[:], -ESHIFT)
            next_row = [12]

            def load_rows():
                lo = next_row[0]
                if lo >= HR:
                    return
                hi = min(lo + 8, HR)
                nc.sync.dma_start(x1_s[:, lo:hi, :], x1c[:, lo:hi, :])
                nc.sync.dma_start(x2_s[:, lo:hi, :], x2c[:, lo:hi, :])
                next_row[0] = hi
            state = {}

            def emit_conv(q):
                """conv3x3 + bias/cast for pair q; prefetch DMAs for pair q."""
                if q % 4 == 0:
                    load_rows()

                xtb_t = xtb_pool.tile([WC, 2, 4, WC + 1], LP_DT)
                nc.sync.dma_start(xtb_t[:], xtb[q])

                q_ps = conv_pp.tile([C, 2, W], F32, tag="q2")
                k_ps = conv_pp.tile([C, 2, W], F32, tag="k2")
                qk = qk_pool.tile([C, 2, 2, W], LP_DT)   # [c, row, q/k, w]
                # all Q taps first: the Q copy starts while K taps stream, and
                # the first pair doesn't stall on the x2 DMA
                for t in range(9):
                    ky, kx = t // 3, t % 3
                    r = 2 * q + ky
                    nc.tensor.matmul(q_ps[:], wt_s[:, t, :],
                                     x1_s[:, r:r + 2, kx:kx + W],
                                     start=(t == 0), stop=(t == 8))
                nc.scalar.activation(qk[:, :, 0, :], q_ps[:], AF.Identity,
                                     bias=bias_s[:], scale=1.0)
                for t in range(9):
                    ky, kx = t // 3, t % 3
                    r = 2 * q + ky
                    nc.tensor.matmul(k_ps[:], wt_s[:, t, :],
                                     x2_s[:, r:r + 2, kx:kx + W],
                                     start=(t == 0), stop=(t == 8))
                nc.scalar.activation(qk[:, :, 1, :], k_ps[:], AF.Identity,
                                     bias=bias_s[:], scale=1.0)
                state[q] = (qk, xtb_t)

            def emit_attn(q):
                """width attention + finalize + store for pair q."""
                qk, xtb_t = state.pop(q)
                out_t = out_pool.tile([WC, 2, 2, 2, C], LP_DT)
                # scores + exp for BOTH rows first: row 1's score matmuls hide
                # row 0's exp latency even when no conv filler is available
                # (pipeline drain at the last pairs).
                e_ts = []
                for rr in range(2):
                    # --- S = Q^T K and St = K^T Q, chunked over W ---
                    s_ps = s_pp.tile([WC, 2, W], F32)
                    st_ps = st_pp.tile([WC, 2, W], F32)
                    for wc in range(2):
                        nc.tensor.matmul(s_ps[:, wc, :],
                                         qk[:, rr, 0, bass.ts(wc, WC)],
                                         qk[:, rr, 1, :],
                                         start=True, stop=True)
                        nc.tensor.matmul(st_ps[:, wc, :],
                                         qk[:, rr, 1, bass.ts(wc, WC)],
                                         qk[:, rr, 0, :],
                                         start=True, stop=True)
                    # --- E = exp(S), Et = exp(St); Et first: the left-apply
                    # (first consumer) reads slots 2,3, so at pipeline drain
                    # its matmuls start while the S exp still runs ---
                    e_t = e_pool.tile([WC, 4, W], LP_DT)
                    nc.scalar.activation(e_t[:, 2:4, :], st_ps[:], AF.Exp,
                                         bias=eshift_s[:])
                    nc.scalar.activation(e_t[:, 0:2, :], s_ps[:], AF.Exp,
                                         bias=eshift_s[:])
                    e_ts.append(e_t)
                for rr in range(2):
                    e_t = e_ts[rr]
                    # --- apply (unnormalized) + ones-column row-sums ---
                    m_ps = m_pp.tile([WC, 4, WC + 1], F32)
                    for wc in range(2):
                        for vc in range(2):
                            nc.tensor.matmul(
                                m_ps[:, wc, :],
                                e_t[:, 2 + vc, bass.ts(wc, WC)],
                                xtb_t[:, rr, 2 + vc, :],
                                start=(vc == 0), stop=(vc == 1))
                    for vc in range(2):
                        for wc in range(2):
                            nc.tensor.matmul(
                                m_ps[:, 2 + vc, :],
                                e_t[:, wc, bass.ts(vc, WC)],
                                xtb_t[:, rr, wc, :],
                                start=(wc == 0), stop=(wc == 1))

                    rcp_t = rcp_pool.tile([WC, 4], F32)
                    nc.vector.reciprocal(rcp_t[:], m_ps[:, :, WC:WC + 1])

                    # --- finalize: out = base^T + M * (1/rs) ---
                    for wc in range(2):
                        nc.vector.scalar_tensor_tensor(
                            out_t[:, 0, rr, wc, :], m_ps[:, wc, 0:C],
                            rcp_t[:, wc:wc + 1], xtb_t[:, rr, wc, 0:C],
                            op0=ALU.mult, op1=ALU.add)
                        nc.vector.scalar_tensor_tensor(
                            out_t[:, 1, rr, wc, :], m_ps[:, 2 + wc, 0:C],
                            rcp_t[:, 2 + wc:3 + wc], xtb_t[:, rr, 2 + wc, 0:C],
                            op0=ALU.mult, op1=ALU.add)
                if q == NPAIR - 1:
                    # last pair: ship row 0 while row 1 finalizes, halving
                    # the final post-compute transfer
                    nc.sync.dma_start(oT[q][:, :, 0, :, :], out_t[:, :, 0, :, :])
                    nc.sync.dma_start(oT[q][:, :, 1, :, :], out_t[:, :, 1, :, :])
                else:
                    nc.sync.dma_start(oT[q], out_t[:])

            # software pipeline: conv runs one pair ahead of attention
            emit_conv(0)
            for q in range(NPAIR):
                if q + 1 < NPAIR:
                    emit_conv(q + 1)
                emit_attn(q)

    nc.compile()
    return nc


def _prepare_inputs(low1, low2, conv_w, conv_b):
    low1 = np.asarray(low1, dtype=np.float32)
    low2 = np.asarray(low2, dtype=np.float32)
    conv_w = np.asarray(conv_w, dtype=np.float32)
    conv_b = np.asarray(conv_b, dtype=np.float32)

    xp1 = np.zeros((B, C, H + 2, W + 2), np.float32)
    xp1[:, :, 1:-1, 1:-1] = low1
    xp2 = np.zeros((B, C, H + 2, W + 2), np.float32)
    xp2[:, :, 1:-1, 1:-1] = low2

    wt = np.ascontiguousarray(
        conv_w.transpose(1, 2, 3, 0).reshape(C, 9, C)).astype(np.float16)
    bias = np.ascontiguousarray(conv_b.reshape(C, 1))

    in_maps = []
    for k in range(NCORES):
        b, half = k // 2, k % 2
        r0 = half * HL

        # padded local row block [C, HL+2, W+2] per tensor
        x1ck = np.ascontiguousarray(
            xp1[b, :, r0:r0 + HL + 2, :]).astype(np.float16)
        x2ck = np.ascontiguousarray(
            xp2[b, :, r0:r0 + HL + 2, :]).astype(np.float16)

        # transposed [h, w', slot, c] for both tensors; slot 0,1=low1T, 2,3=low2T
        l1t = low1[b, :, r0:r0 + HL, :].transpose(1, 2, 0)   # [h, w, c]
        l2t = low2[b, :, r0:r0 + HL, :].transpose(1, 2, 0)
        a1 = l1t.reshape(HL, 2, WC, C).transpose(0, 2, 1, 3)  # [h, w', wc, c]
        a2 = l2t.reshape(HL, 2, WC, C).transpose(0, 2, 1, 3)
        xt = np.concatenate([a1, a2], axis=2)                 # [h, w', 4, c]
        # pair-batch: [pair, w', row, slot, c] + ones column
        xt32 = xt.reshape(NPAIR, 2, WC, 4, C).transpose(0, 2, 1, 3, 4)
        xtb = np.concatenate(
            [xt32, np.ones((NPAIR, WC, 2, 4, 1), np.float32)],
            axis=4).astype(np.float16)

        in_maps.append({
            "x1c": x1ck,
            "x2c": x2ck,
            "xtb": np.ascontiguousarray(xtb),
            "wt": wt,
            "bias": bias,
        })
    return in_maps


def _assemble(results):
    left = np.empty((B, C, H, W), np.float32)
    right = np.empty((B, C, H, W), np.float32)
    for k in range(NCORES):
        b, half = k // 2, k % 2
        r0 = half * HL
        arr = results[k]["oT"].astype(np.float32)  # [pair, p, side, rr, wc, c]
        for side, dst in ((0, left), (1, right)):
            # -> [c, pair, rr, wc, p] -> [c, h, w]
            dst[b, :, r0:r0 + HL, :] = (
                arr[:, :, side].transpose(4, 0, 2, 3, 1).reshape(C, HL, W))
    return left, right


def _run(inputs, trace=False):
    if trace:
        _install_profile_hook()
    if "nc" not in _CACHE:
        _CACHE["nc"] = _build()
    nc = _CACHE["nc"]
    in_maps = _prepare_inputs(**inputs)
    res = bass_utils.run_bass_kernel_spmd(
        nc, in_maps, core_ids=list(range(NCORES)), trace=trace)
    left, right = _assemble(res.results)
    return (left, right), res


def kernel(**inputs):
    out, _ = _run(inputs, trace=False)
    return out


# revision 25
# speedup vs baseline: 1.1990x; 1.0039x over previous
"""Trainium2 Bass kernel for width-axis cross attention (sparse_attention problem).

reference semantics:
  Q = conv3x3(low1, w, b); K = conv3x3(low2, w, b)
  score[b,h,w,v] = sum_c Q[b,c,h,w] * K[b,c,h,v]
  A_left  = softmax(score, axis=-1)            (relu is identity on softmax)
  A_right = softmax(score^T, axis=-1)
  left  = low1 + einsum('bhwv,bchv->bchw', A_left,  low2)
  right = low2 + einsum('bhwv,bchv->bchw', A_right, low1)

Sharding: data-parallel over (batch, H-half) -> 8 shards, no cross-core comm.

Per-core dataflow (96 rows, processed in row pairs), fp16 matmul lanes /
fp32 accumulation+normalization:
 - conv as 9 accumulating fp16 matmuls per tensor, 2 output rows per matmul
   (N=384; weight loads amortized and hidden), PSUM -> SBUF fp16 with bias
   via ScalarE.
 - S = Q^T K and St = K^T Q in fp16; exp(S - 12) via ScalarE (one [96,384]
   op per side; the constant shift keeps unnormalized exp inside fp16 range
   and cancels in the softmax normalization).
 - apply matmuls in fp16 against host-pre-transposed inputs, with an extra
   all-ones column producing the softmax row-sums for free (column sums of
   exp land in PSUM column 96).
 - finalize = (M * 1/rs) + base^T in one fused VectorE scalar_tensor_tensor
   op; the fp16 base is re-read from the same xtb tile the apply used, and
   the output is stored width-transposed in fp16 as one tensor (single DMA
   per pair); host un-transposes.
 - the whole padded input block stays resident in SBUF (loaded once in
   row slices, first pair's rows first) -- no per-chunk halo re-reads.

Measured on 8 axon-tunneled TRN2 cores at full clock: HW exec ~231 us
(tensor engine >99% busy at the fp16 streaming bound for its 480k matmul
rows; fp8 fails the accuracy gate, DMA-XBAR transposes trip the activity
throttle), scale-relative max error ~1.4e-3 vs the fp32 reference.
"""

import os
import sys

for _p in ("/opt/trn_rl_repo", "/root/.axon_site/_ro/trn_rl_repo"):
    if os.path.isdir(_p) and _p not in sys.path:
        sys.path.append(_p)

import numpy as np

import concourse.bacc as bacc
import concourse.bass as bass
import concourse.tile as tile
from concourse import mybir
from concourse import bass_utils

B, C, H, W = 4, 96, 192, 192
NCORES = 8
HL = H // 2          # local rows per core
WP = W + 2           # width-padded
WC = W // 2          # 96-wide chunk of the W axis
NPAIR = HL // 2      # 48 row pairs


F32 = mybir.dt.float32
FP16 = mybir.dt.float16
AF = mybir.ActivationFunctionType
ALU = mybir.AluOpType

LP_DT, LP_NP = FP16, np.float16
ESHIFT = 12.0

_CACHE = {}


def _install_profile_hook():
    """Register the axon NTFF profiling hook (missing from this image's antenv)."""
    if _CACHE.get("hook_done"):
        return
    _CACHE["hook_done"] = True
    import types
    import antenv

    if "antenv.axon_hooks" not in sys.modules:
        mod = types.ModuleType("antenv.axon_hooks")
        _h = {"fn": None}
        mod.set_axon_ntff_profile_hook = lambda fn: _h.__setitem__("fn", fn)
        mod.get_axon_ntff_profile_hook = lambda: _h["fn"]
        sys.modules["antenv.axon_hooks"] = mod
        antenv.axon_hooks = mod
    mod = sys.modules["antenv.axon_hooks"]
    try:
        from trn_agent_boot.trn_boot import _ntff_profile_via_ctypes

        hook = _ntff_profile_via_ctypes("/opt/axon/libaxon_pjrt.so")
        if hook is not None:
            mod.set_axon_ntff_profile_hook(hook)
    except Exception as e:  # profiling is best-effort
        print(f"profile hook install failed: {e}", file=sys.stderr)
    # avoid remote artifact uploads from the profiling path
    bass_utils.upload_artifacts = lambda tmpdir: "local://" + str(tmpdir)


def _build():
    """Build + compile the per-core Bass module (identical on all 8 cores)."""
    nc = bacc.Bacc("TRN2", target_bir_lowering=False, debug=False,
                   num_devices=NCORES)

    # inputs (per core); x1/x2 are the padded local row block, loaded once
    # (no per-chunk halo duplication)
    HR = HL + 2   # 98 padded rows resident per tensor
    x1c = nc.dram_tensor("x1c", [C, HR, WP], LP_DT,
                         kind="ExternalInput").ap()
    x2c = nc.dram_tensor("x2c", [C, HR, WP], LP_DT,
                         kind="ExternalInput").ap()
    # [pair, w', row, slot, col]; slots 0,1 = low1T w-chunks, 2,3 = low2T
    # xtb has a 97th all-ones column for free softmax row-sums.
    xtb = nc.dram_tensor("xtb", [NPAIR, WC, 2, 4, WC + 1], LP_DT,
                         kind="ExternalInput").ap()
    wt = nc.dram_tensor("wt", [C, 9, C], LP_DT, kind="ExternalInput").ap()
    bias = nc.dram_tensor("bias", [C, 1], F32, kind="ExternalInput").ap()
    # merged output, transposed layout: [pair, part, side, row, chunk, c]
    oT = nc.dram_tensor("oT", [NPAIR, WC, 2, 2, 2, C], LP_DT,
                        kind="ExternalOutput").ap()

    with tile.TileContext(nc) as tc:
        with (
            tc.tile_pool(name="wpool", bufs=1) as wpool,
            tc.tile_pool(name="xres", bufs=1) as x_pool,
            tc.tile_pool(name="xtbp", bufs=3) as xtb_pool,
            tc.tile_pool(name="qkp", bufs=2) as qk_pool,
            tc.tile_pool(name="ep", bufs=3) as e_pool,
            tc.tile_pool(name="rcpp", bufs=3) as rcp_pool,
            tc.tile_pool(name="outp", bufs=3) as out_pool,
            tc.tile_pool(name="convps", bufs=1, space="PSUM") as conv_pp,
            tc.tile_pool(name="sps", bufs=2, space="PSUM") as s_pp,
            tc.tile_pool(name="mps", bufs=2, space="PSUM") as m_pp,
        ):
            # whole padded input block resident in SBUF (38 KB/partition per
            # tensor) -- no per-chunk halo re-reads
            x1_s = x_pool.tile([C, HR, WP], LP_DT)
            x2_s = x_pool.tile([C, HR, WP], LP_DT)
            # first pair's rows land first so the first conv matmul starts
            # as early as possible; dispatch cost (~600ns each) means fewer,
            # earlier DMAs beat channel-splitting here
            nc.sync.dma_start(x1_s[:, 0:4, :], x1c[:, 0:4, :])
            wt_s = wpool.tile([C, 9, C], LP_DT)
            nc.sync.dma_start(wt_s[:], wt)
            nc.sync.dma_start(x2_s[:, 0:4, :], x2c[:, 0:4, :])
            bias_s = wpool.tile([C, 1], F32)
            nc.sync.dma_start(bias_s[:], bias)
            nc.sync.dma_start(x1_s[:, 4:12, :], x1c[:, 4:12, :])
            nc.sync.dma_start(x2_s[:, 4:12, :], x2c[:, 4:12, :])
            eshift_s = wpool.tile([WC, 1], F32)
            nc.gpsimd.memset(eshift_s[:], -ESHIFT)
            next_row = [12]

            def load_rows():
                lo = next_row[0]
                if lo >= HR:
                    return
                hi = min(lo + 8, HR)
                nc.sync.dma_start(x1_s[:, lo:hi, :], x1c[:, lo:hi, :])
                nc.sync.dma_start(x2_s[:, lo:hi, :], x2c[:, lo:hi, :])
                next_row[0] = hi
            state = {}

            def emit_conv(q):
                """conv3x3 + bias/cast for pair q; prefetch DMAs for pair q."""
                if q % 4 == 0:
                    load_rows()

                xtb_t = xtb_pool.tile([WC, 2, 4, WC + 1], LP_DT)
                nc.sync.dma_start(xtb_t[:], xtb[q])

                q_ps = conv_pp.tile([C, 2, W], F32, tag="q2")
                k_ps = conv_pp.tile([C, 2, W], F32, tag="k2")
                qk = qk_pool.tile([C, 2, 2, W], LP_DT)   # [c, row, q/k, w]
                # all Q taps first: the Q copy starts while K taps stream, and
                # the first pair doesn't stall on the x2 DMA
                for t in range(9):
                    ky, kx = t // 3, t % 3
                    r = 2 * q + ky
                    nc.tensor.matmul(q_ps[:], wt_s[:, t, :],
                                     x1_s[:, r:r + 2, kx:kx + W],
                                     start=(t == 0), stop=(t == 8))
                nc.scalar.activation(qk[:, :, 0, :], q_ps[:], AF.Identity,
                                     bias=bias_s[:], scale=1.0)
                for t in range(9):
                    ky, kx = t // 3, t % 3
                    r = 2 * q + ky
                    nc.tensor.matmul(k_ps[:], wt_s[:, t, :],
                                     x2_s[:, r:r + 2, kx:kx + W],
                                     start=(t == 0), stop=(t == 8))
                nc.scalar.activation(qk[:, :, 1, :], k_ps[:], AF.Identity,
                                     bias=bias_s[:], scale=1.0)
                state[q] = (qk, xtb_t)

            def emit_attn(q):
                """width attention + finalize + store for pair q."""
                qk, xtb_t = state.pop(q)
                out_t = out_pool.tile([WC, 2, 2, 2, C], LP_DT)
                # scores + exp for BOTH rows first: row 1's score matmuls hide
                # row 0's exp latency even when no conv filler is available
                # (pipeline drain at the last pairs).
                e_ts = []
                for rr in range(2):
                    # --- S = Q^T K and St = K^T Q, chunked over W; one
                    # bank-aligned PSUM tile (256-col pitch keeps every
                    # matmul inside a 2KB bank) so ONE exp covers all 4
                    # slots: fewer scalar ops, shorter drain chain ---
                    s_ps = s_pp.tile([WC, 4, 256], F32)
                    for wc in range(2):
                        nc.tensor.matmul(s_ps[:, wc, 0:W],
                                         qk[:, rr, 0, bass.ts(wc, WC)],
                                         qk[:, rr, 1, :],
                                         start=True, stop=True)
                        nc.tensor.matmul(s_ps[:, 2 + wc, 0:W],
                                         qk[:, rr, 1, bass.ts(wc, WC)],
                                         qk[:, rr, 0, :],
                                         start=True, stop=True)
                    # --- E = exp(S) / exp(St), single activation ---
                    e_t = e_pool.tile([WC, 4, W], LP_DT)
                    nc.scalar.activation(e_t[:], s_ps[:, :, 0:W], AF.Exp,
                                         bias=eshift_s[:])
                    e_ts.append(e_t)
                for rr in range(2):
                    e_t = e_ts[rr]
                    # --- apply (unnormalized) + ones-column row-sums ---
                    m_ps = m_pp.tile([WC, 4, WC + 1], F32)
                    for wc in range(2):
                        for vc in range(2):
                            nc.tensor.matmul(
                                m_ps[:, wc, :],
                                e_t[:, 2 + vc, bass.ts(wc, WC)],
                                xtb_t[:, rr, 2 + vc, :],
                                start=(vc == 0), stop=(vc == 1))
                    for vc in range(2):
                        for wc in range(2):
                            nc.tensor.matmul(
                                m_ps[:, 2 + vc, :],
                                e_t[:, wc, bass.ts(vc, WC)],
                                xtb_t[:, rr, wc, :],
                                start=(wc == 0), stop=(wc == 1))

                    rcp_t = rcp_pool.tile([WC, 4], F32)
                    nc.vector.reciprocal(rcp_t[:], m_ps[:, :, WC:WC + 1])

                    # --- finalize: out = base^T + M * (1/rs) ---
                    for wc in range(2):
                        nc.vector.scalar_tensor_tensor(
                            out_t[:, 0, rr, wc, :], m_ps[:, wc, 0:C],
                            rcp_t[:, wc:wc + 1], xtb_t[:, rr, wc, 0:C],
                            op0=ALU.mult, op1=ALU.add)
                        nc.vector.scalar_tensor_tensor(
                            out_t[:, 1, rr, wc, :], m_ps[:, 2 + wc, 0:C],
                            rcp_t[:, 2 + wc:3 + wc], xtb_t[:, rr, 2 + wc, 0:C],
                            op0=ALU.mult, op1=ALU.add)
                if q == NPAIR - 1:
                    # last pair: ship row 0 while row 1 finalizes, halving
                    # the final post-compute transfer
                    nc.sync.dma_start(oT[q][:, :, 0, :, :], out_t[:, :, 0, :, :])
                    nc.sync.dma_start(oT[q][:, :, 1, :, :], out_t[:, :, 1, :, :])
                else:
                    nc.sync.dma_start(oT[q], out_t[:])

            # software pipeline: conv runs one pair ahead of attention
            emit_conv(0)
            for q in range(NPAIR):
                if q + 1 < NPAIR:
                    emit_conv(q + 1)
                emit_attn(q)

    nc.compile()
    return nc


def _prepare_inputs(low1, low2, conv_w, conv_b):
    low1 = np.asarray(low1, dtype=np.float32)
    low2 = np.asarray(low2, dtype=np.float32)
    conv_w = np.asarray(conv_w, dtype=np.float32)
    conv_b = np.asarray(conv_b, dtype=np.float32)

    xp1 = np.zeros((B, C, H + 2, W + 2), np.float32)
    xp1[:, :, 1:-1, 1:-1] = low1
    xp2 = np.zeros((B, C, H + 2, W + 2), np.float32)
    xp2[:, :, 1:-1, 1:-1] = low2

    wt = np.ascontiguousarray(
        conv_w.transpose(1, 2, 3, 0).reshape(C, 9, C)).astype(np.float16)
    bias = np.ascontiguousarray(conv_b.reshape(C, 1))

    in_maps = []
    for k in range(NCORES):
        b, half = k // 2, k % 2
        r0 = half * HL

        # padded local row block [C, HL+2, W+2] per tensor
        x1ck = np.ascontiguousarray(
            xp1[b, :, r0:r0 + HL + 2, :]).astype(np.float16)
        x2ck = np.ascontiguousarray(
            xp2[b, :, r0:r0 + HL + 2, :]).astype(np.float16)

        # transposed [h, w', slot, c] for both tensors; slot 0,1=low1T, 2,3=low2T
        l1t = low1[b, :, r0:r0 + HL, :].transpose(1, 2, 0)   # [h, w, c]
        l2t = low2[b, :, r0:r0 + HL, :].transpose(1, 2, 0)
        a1 = l1t.reshape(HL, 2, WC, C).transpose(0, 2, 1, 3)  # [h, w', wc, c]
        a2 = l2t.reshape(HL, 2, WC, C).transpose(0, 2, 1, 3)
        xt = np.concatenate([a1, a2], axis=2)                 # [h, w', 4, c]
        # pair-batch: [pair, w', row, slot, c] + ones column
        xt32 = xt.reshape(NPAIR, 2, WC, 4, C).transpose(0, 2, 1, 3, 4)
        xtb = np.concatenate(
            [xt32, np.ones((NPAIR, WC, 2, 4, 1), np.float32)],
            axis=4).astype(np.float16)

        in_maps.append({
            "x1c": x1ck,
            "x2c": x2ck,
            "xtb": np.ascontiguousarray(xtb),
            "wt": wt,
            "bias": bias,
        })
    return in_maps


def _assemble(results):
    left = np.empty((B, C, H, W), np.float32)
    right = np.empty((B, C, H, W), np.float32)
    for k in range(NCORES):
        b, half = k // 2, k % 2
        r0 = half * HL
        arr = results[k]["oT"].astype(np.float32)  # [pair, p, side, rr, wc, c]
        for side, dst in ((0, left), (1, right)):
            # -> [c, pair, rr, wc, p] -> [c, h, w]
            dst[b, :, r0:r0 + HL, :] = (
                arr[:, :, side].transpose(4, 0, 2, 3, 1).reshape(C, HL, W))
    return left, right


def _run(inputs, trace=False):
    if trace:
        _install_profile_hook()
    if "nc" not in _CACHE:
        _CACHE["nc"] = _build()
    nc = _CACHE["nc"]
    in_maps = _prepare_inputs(**inputs)
    res = bass_utils.run_bass_kernel_spmd(
        nc, in_maps, core_ids=list(range(NCORES)), trace=trace)
    left, right = _assemble(res.results)
    return (left, right), res


def kernel(**inputs):
    out, _ = _run(inputs, trace=False)
    return out
